# revision 2
# baseline (speedup 1.0000x reference)
"""Trainium2 Bass kernel v2 for nn_AttnLayer_71382356460296.

Sharding: data-parallel over batch B (2) x cyclic query-tile-parallel (4)
=> 8 cores. Each core takes 8 interleaved 128-query tiles (global tile
g = c + 4*j for local j=0..7), so every core owns exactly NMASK masked
(triangular-zone) tiles and the causal mask is pure host DATA -- one
SPMD module for all cores.

Per-core pipeline (pair = 2 query heads sharing layout, 16 pairs):
  qproj (PE, dense bf16) -> de-interleaved RoPE (DVE only, no shuffle)
  -> row-packed K=64 scores (2 heads concurrent via tile_position)
  -> exp on ACT over [128, 2*HW] merged tiles -> mask muls (gpsimd, data)
  -> AV with denominator replicated across 64 partitions (no broadcast
  matmul) -> reciprocal_approx_fast + one mul per head -> bf16 at tiles
  -> output projection streamed vs prefetched Wo chunks.
"""

import time

import numpy as np

import concourse.bacc as bacc
import concourse.mybir as mybir
import concourse.tile as tile
from concourse.bass_utils import run_bass_kernel_spmd

F32 = mybir.dt.float32
BF16 = mybir.dt.bfloat16
AF = mybir.ActivationFunctionType

FULL = dict(B=2, T=4096, D=2048, H=32, KV=8, DH=64, W=1024, BASE=10000.0)


def _derived(cfg):
    d = dict(cfg)
    d["CH"] = cfg["T"] // 4                   # queries per core
    d["QT"] = d["CH"] // 128                  # local query tiles
    d["KB"] = cfg["W"] // 128                 # key blocks in window
    d["DT"] = cfg["D"] // 128                 # contraction tiles for Wq
    d["NP"] = cfg["H"] // 2                   # head pairs
    d["WC"] = max(1, cfg["D"] // 512)         # 512-wide weight chunks
    d["HW_"] = d["CH"] // 2                   # query columns per half
    d["NM"] = -(-d["KB"] // 4)                # masked local qtiles (ceil)
    d["PPW"] = d["NP"] // d["WC"]             # pairs per wq chunk
    assert d["NP"] * 128 == cfg["D"]
    assert d["NM"] * 128 <= d["HW_"]
    return d


def build(cfg, pack_scores=True):
    c = _derived(cfg)
    CH, KB, DT, NP, KV, H = c["CH"], c["KB"], c["DT"], c["NP"], c["KV"], c["H"]
    HW_, NM, WC, PPW = c["HW_"], c["NM"], c["WC"], c["PPW"]
    # concurrent row-tiled score MMs write spp[:, :HW_] / spp[:, HW_:];
    # they must land in different PSUM banks (HW fault otherwise)
    pack_scores = pack_scores and (HW_ * 4 >= 2048)
    MW = NM * 128                              # masked columns per head
    hpkv = H // KV
    LOOK = 3                                   # score lookahead in kb loop
    nc = bacc.Bacc("TRN2", target_bir_lowering=False, debug=False)

    # Host-rearranged inputs (layouts documented in host_inputs):
    xr = nc.dram_tensor("xr", [128, DT, CH], BF16, kind="ExternalInput")
    wqr = nc.dram_tensor("wqr", [128, WC, DT, 512], BF16, kind="ExternalInput")
    wor = nc.dram_tensor("wor", [128, WC, NP, 512], BF16, kind="ExternalInput")
    kTd = nc.dram_tensor("kTd", [128, KV, KB, 128], BF16, kind="ExternalInput")
    kT2 = nc.dram_tensor("kT2", [128, KV, KB, 2, 128], BF16,
                         kind="ExternalInput")
    vad = nc.dram_tensor("vad", [128, KV, KB, 128], BF16, kind="ExternalInput")
    cosT = nc.dram_tensor("cosT", [128, CH], BF16, kind="ExternalInput")
    sinT = nc.dram_tensor("sinT", [128, CH], BF16, kind="ExternalInput")
    mskT = nc.dram_tensor("mskT", [128, KB, MW], BF16, kind="ExternalInput")
    out = nc.dram_tensor("out", [CH, cfg["D"]], F32, kind="ExternalOutput")

    with nc.allow_low_precision(reason="bf16 matmuls are intended"), \
         tile.TileContext(nc) as tc:
        with (
            tc.tile_pool(name="consts", bufs=1) as cp,
            tc.tile_pool(name="xp", bufs=1) as xp,
            tc.tile_pool(name="wpool", bufs=3) as wp,
            tc.tile_pool(name="qtp", bufs=6) as qtp,
            tc.tile_pool(name="ropep", bufs=2) as rp,
            tc.tile_pool(name="erp", bufs=8) as ep,
            tc.tile_pool(name="rcpp", bufs=4) as rcp,
            tc.tile_pool(name="atp", bufs=1) as atp,
            tc.tile_pool(name="ps", bufs=2, space="PSUM") as ps,
        ):
            # ---- input DMAs, priority order (one HWDGE queue, FIFO).
            # Interleave wq chunk0 + x by kt-group so pair-0 qproj starts
            # as soon as the first slices land (subtile deps).
            xts = xp.tile([128, DT, CH], BF16)
            wqs = []
            wq0 = wp.tile([128, DT, 512], BF16, tag="w")
            wqs.append(wq0)
            kslc = max(1, DT // 4)
            for kt0 in range(0, DT, kslc):
                nc.sync.dma_start(wq0[:, kt0:kt0 + kslc],
                                  wqr[:, 0, kt0:kt0 + kslc])
                nc.sync.dma_start(xts[:, kt0:kt0 + kslc], xr[:, kt0:kt0 + kslc])
            cos_sb = cp.tile([128, CH], BF16)
            nc.sync.dma_start(cos_sb[:], cosT[:])
            sin_sb = cp.tile([128, CH], BF16)
            nc.sync.dma_start(sin_sb[:], sinT[:])
            if pack_scores:
                kt_sb = cp.tile([128, KV, KB, 128], BF16)
            else:
                kt2_sb = cp.tile([128, KV, KB, 2, 128], BF16)
            va_sb = cp.tile([128, KV, KB, 128], BF16)
            msk_sb = cp.tile([128, KB, MW], BF16)
            nc.sync.dma_start(msk_sb[:], mskT[:])
            # K/V per kv-head, in first-use order (pair m uses g = m//2)
            for g in range(KV):
                if pack_scores:
                    nc.sync.dma_start(kt_sb[:, g], kTd[:, g])
                else:
                    nc.sync.dma_start(kt2_sb[:, g], kT2[:, g])
                nc.sync.dma_start(va_sb[:, g], vad[:, g])
            for wc in range(1, WC):
                wq_c = wp.tile([128, DT, 512], BF16, tag="w")
                nc.sync.dma_start(wq_c[:], wqr[:, wc])
                wqs.append(wq_c)
            wos = []

            def _rope(m, h, qp, qt):
                """de-interleaved rope (DVE only): qt = qp*cos + swap32(qp)*sin"""
                n0 = HW_ * h
                t1 = rp.tile([128, HW_], F32, tag="t1", name=f"t1_{m}{h}")
                nc.vector.tensor_mul(t1[:], qp[:], cos_sb[:, n0:n0 + HW_])
                t2 = rp.tile([128, HW_], F32, tag="t2", name=f"t2_{m}{h}")
                for blk in range(4):
                    dst = slice(32 * blk, 32 * blk + 32)
                    src = slice(32 * (blk ^ 1), 32 * (blk ^ 1) + 32)
                    nc.vector.tensor_mul(
                        t2[dst], qp[src], sin_sb[dst, n0:n0 + HW_])
                nc.vector.tensor_add(qt[:], t1[:], t2[:])

            def qproj_fillers(m):
                """Pair-m qproj as a list of single-MM closures (+ rope at
                each half's end); returns (fillers, qt halves)."""
                wq_c = wqs[m // PPW]
                me = 128 * (m % PPW)
                qts = [qtp.tile([128, HW_], BF16, tag="qt", name=f"qt{m}_{h}")
                       for h in range(2)]
                fillers = []
                for h in range(2):
                    qp = ps.tile([128, HW_], F32, tag="qp", bufs=2,
                                 name=f"qp{m}_{h}")
                    n0 = HW_ * h

                    def mk(kt, qp=qp, n0=n0, h=h):
                        def emit():
                            nc.tensor.matmul(
                                qp[:], wq_c[:, kt, me:me + 128],
                                xts[:, kt, n0:n0 + HW_],
                                start=(kt == 0), stop=(kt == DT - 1))
                            if kt == DT - 1:
                                _rope(m, h, qp, qts[h])
                        return emit

                    fillers.extend(mk(kt) for kt in range(DT))
                return fillers, qts

            def qproj_rope(m):
                """Non-pipelined variant (prologue): emit everything now."""
                fillers, qts = qproj_fillers(m)
                for f in fillers:
                    f()
                return qts

            def attn_half(m, h, qt_h, avs, fillers):
                """scores -> exp -> mask -> AV for one column half.
                kb processed in groups of 2 to halve 64<->128 tiling-mode
                switches; up to 4 filler MMs (qproj of pair m+2) per group."""
                g = (2 * m) // hpkv
                av0, av1 = avs
                ers = []
                NG = -(-KB // 2)
                LOOKG = 3
                for step in range(NG + LOOKG):
                    for kb in range(2 * step, min(2 * step + 2, KB)):
                        spp = ps.tile([128, 2 * HW_], F32, tag="spp", bufs=2,
                                      name=f"sp{m}_{h}_{kb}")
                        if pack_scores:
                            nc.tensor.matmul(
                                spp[:, 0:HW_], kt_sb[0:64, g, kb, :],
                                qt_h[0:64, :], start=True, stop=True,
                                tile_position=(0, 0))
                            nc.tensor.matmul(
                                spp[:, HW_:2 * HW_], kt_sb[64:128, g, kb, :],
                                qt_h[64:128, :], start=True, stop=True,
                                tile_position=(64, 0))
                        else:
                            nc.tensor.matmul(
                                spp[:, 0:HW_], kt2_sb[:, g, kb, 0, :],
                                qt_h[:], start=True, stop=True)
                            nc.tensor.matmul(
                                spp[:, HW_:2 * HW_], kt2_sb[:, g, kb, 1, :],
                                qt_h[:], start=True, stop=True)
                        er = ep.tile([128, 2 * HW_], BF16, tag="er",
                                     name=f"er{m}_{h}_{kb}")
                        nc.scalar.activation(er[:], spp[:], AF.Exp)
                        if h == 0:
                            nc.vector.tensor_mul(
                                er[:, 0:MW], er[:, 0:MW], msk_sb[:, kb, :])
                            nc.gpsimd.tensor_mul(
                                er[:, HW_:HW_ + MW], er[:, HW_:HW_ + MW],
                                msk_sb[:, kb, :])
                        ers.append(er)
                    for _ in range(4):
                        if fillers:
                            fillers.pop(0)()
                    for kb in range(2 * (step - LOOKG),
                                    min(2 * (step - LOOKG) + 2, KB)):
                        if kb < 0:
                            continue
                        er = ers[kb]
                        nc.tensor.matmul(
                            av0[:], va_sb[:, g, kb, :], er[:, 0:HW_],
                            start=(kb == 0), stop=(kb == KB - 1))
                        nc.tensor.matmul(
                            av1[:], va_sb[:, g, kb, :], er[:, HW_:2 * HW_],
                            start=(kb == 0), stop=(kb == KB - 1))

            def normalize(m, h, avs, at_m):
                av0, av1 = avs
                n0 = HW_ * h
                ra = rcp.tile([64, HW_], F32, tag="rcp", name=f"ra{m}{h}")
                nc.vector.reciprocal_approx_fast(ra[:], av0[0:64, :])
                nc.vector.tensor_mul(at_m[0:64, n0:n0 + HW_],
                                     av0[64:128, :], ra[:])
                rb = rcp.tile([64, HW_], F32, tag="rcp", name=f"rb{m}{h}")
                nc.vector.reciprocal_approx_fast(rb[:], av1[0:64, :])
                nc.vector.tensor_mul(at_m[64:128, n0:n0 + HW_],
                                     av1[64:128, :], rb[:])

            # ---- software-pipelined pair loop: iter i runs attention of
            # pair i and qproj+rope of pair i+1.
            ats = [atp.tile([128, CH], BF16, tag=f"at{m}", name=f"at{m}")
                   for m in range(NP)]
            qt_q = [qproj_rope(0), qproj_rope(1) if NP > 1 else None]
            fillers = []
            for i in range(NP):
                qts = qt_q[0]
                if i + 2 < NP:
                    fl, nqts = qproj_fillers(i + 2)
                    fillers.extend(fl)
                else:
                    nqts = None
                avh0 = [ps.tile([128, HW_], F32, tag="av", bufs=2,
                                name=f"av0h0_{i}"),
                        ps.tile([128, HW_], F32, tag="av", bufs=2,
                                name=f"av1h0_{i}")]
                attn_half(i, 0, qts[0], avh0, fillers)
                normalize(i, 0, avh0, ats[i])
                # prefetch Wo chunk w once wq chunk w is dead
                if i % PPW == PPW - 1 and len(wos) < WC:
                    wo_c = wp.tile([128, NP, 512], BF16, tag="w",
                                   name=f"wo{len(wos)}")
                    nc.sync.dma_start(wo_c[:], wor[:, len(wos)])
                    wos.append(wo_c)
                avh1 = [ps.tile([128, HW_], F32, tag="av", bufs=2,
                                name=f"av0h1_{i}"),
                        ps.tile([128, HW_], F32, tag="av", bufs=2,
                                name=f"av1h1_{i}")]
                attn_half(i, 1, qts[1], avh1, fillers)
                normalize(i, 1, avh1, ats[i])
                qt_q = [qt_q[1], nqts]
            while fillers:
                fillers.pop(0)()

        # ---- phase C: out[q, o] = sum_m at_m.T @ woT_m
        with (
            tc.tile_pool(name="osb", bufs=3) as op_,
            tc.tile_pool(name="psc", bufs=4, space="PSUM") as psc,
        ):
            MQ = CH // 128
            for oc in range(WC):
                wo_c = wos[oc]
                for mq in range(MQ):
                    opx = psc.tile([128, 512], F32, tag="opx",
                                   name=f"opx{oc}_{mq}")
                    for kq in range(NP):
                        nc.tensor.matmul(
                            opx[:],
                            ats[kq][:, 128 * mq:128 * (mq + 1)],
                            wo_c[:, kq, :],
                            start=(kq == 0), stop=(kq == NP - 1))
                    osb = op_.tile([128, 512], F32, tag="os",
                                   name=f"osb{oc}_{mq}")
                    if mq % 2 == 0:
                        nc.scalar.copy(osb[:], opx[:])
                    else:
                        nc.vector.tensor_copy(osb[:], opx[:])
                    nc.sync.dma_start(
                        out[128 * mq:128 * (mq + 1), 512 * oc:512 * (oc + 1)],
                        osb[:])
    nc.compile()
    return nc


def host_inputs(cfg, x, k_cache, v_cache, Wq, Wo, core):
    import ml_dtypes
    c = _derived(cfg)
    CH, KB, KV, W, DH = c["CH"], c["KB"], c["KV"], c["W"], c["DH"]
    QT, DT, NP, D, WC, HW_, NM = (c["QT"], c["DT"], c["NP"], c["D"], c["WC"],
                                  c["HW_"], c["NM"])
    H = cfg["H"]
    b, cc = core // 4, core % 4
    Tc = k_cache.shape[2]
    f32 = np.float32
    bf16 = ml_dtypes.bfloat16

    # de-interleave permutation within a 64-slot head block:
    s = np.arange(64)
    dperm = np.where(s < 32, 2 * s, 2 * (s - 32) + 1)       # slot -> dim

    gtiles = cc + 4 * np.arange(QT)                          # global qtiles
    qidx = (128 * gtiles[:, None] + np.arange(128)[None, :]).reshape(-1)

    xT = x[b, qidx, :].T                                     # (D, CH)
    xr = np.ascontiguousarray(
        xT.reshape(DT, 128, CH).transpose(1, 0, 2)).astype(bf16)

    wqT = Wq.T * f32(1.0 / np.sqrt(DH))                      # (D, H*DH)
    # column permutation: pair m block col mp -> q dim 128m + perm128[mp]
    perm128 = np.concatenate([dperm, 64 + dperm])            # (128,)
    cols = np.concatenate([128 * m + perm128 for m in range(NP)])
    wqp = wqT[:, cols]                                       # (D, D) permuted
    wqr = np.ascontiguousarray(
        wqp.reshape(DT, 128, WC, 512).transpose(1, 2, 0, 3)).astype(bf16)

    woT = Wo.T                                               # (D, D)
    wor = np.ascontiguousarray(
        woT.reshape(NP, 128, WC, 512).transpose(1, 2, 0, 3)).astype(bf16)

    kw = k_cache[b, :, Tc - W:, :]                           # (KV, W, DH)
    kTd = np.empty((128, KV, KB, 128), f32)
    kblk = kw.reshape(KV, KB, 128, DH).transpose(3, 0, 1, 2)  # (DH,KV,KB,128)
    kTd[0:64] = kblk[dperm]
    kTd[64:128] = kblk[dperm]
    kT2 = np.zeros((128, KV, KB, 2, 128), f32)
    kT2[0:64, :, :, 0, :] = kblk[dperm]
    kT2[64:128, :, :, 1, :] = kblk[dperm]

    vw = v_cache[b, :, Tc - W:, :].reshape(KV, KB, 128, DH)
    vad = np.empty((128, KV, KB, 128), f32)
    vad[:, :, :, 0:64] = 1.0                                 # den replicate
    vad[:, :, :, 64:128] = vw.transpose(2, 0, 1, 3)

    pos = qidx.astype(f32)                                   # global position
    inv = 1.0 / (cfg["BASE"] ** (2.0 * np.arange(32) / DH))
    r = np.arange(128)
    ang = pos[None, :] * inv[r % 32][:, None]                # (128, CH)
    cosT = np.cos(ang).astype(bf16)
    sgn = np.where((r % 64) < 32, -1.0, 1.0)[:, None]
    sinT = (np.sin(ang) * sgn).astype(bf16)

    # masks for local qtiles 0..NM-1: key 128kb+kr visible to query
    # 128*gtiles[j]+qr iff key_idx <= query_idx
    mskT = np.zeros((128, KB, NM * 128), f32)
    for j in range(NM):
        gq = 128 * gtiles[j] + np.arange(128)                # (128,) queries
        for kb in range(KB):
            kk = 128 * kb + np.arange(128)                   # (128,) keys
            mskT[:, kb, 128 * j:128 * (j + 1)] = (
                kk[:, None] <= gq[None, :])
    return {"xr": xr, "wqr": wqr, "wor": wor, "kTd": kTd.astype(bf16),
            "kT2": kT2.astype(bf16), "vad": vad.astype(bf16), "cosT": cosT,
            "sinT": sinT, "mskT": mskT.astype(bf16)}


_NC_CACHE = {}


def _assemble(cfg, outs):
    c = _derived(cfg)
    CH, QT, D, B, T = c["CH"], c["QT"], c["D"], cfg["B"], cfg["T"]
    full = np.empty((B, T, D), np.float32)
    for core in range(8):
        b, cc = core // 4, core % 4
        gtiles = cc + 4 * np.arange(QT)
        qidx = (128 * gtiles[:, None] + np.arange(128)[None, :]).reshape(-1)
        full[b, qidx, :] = outs[core]
    return full


def run(cfg, x, k_cache, v_cache, Wq, Wo, trace=False, pack_scores=True):
    pack_scores = pack_scores and (_derived(cfg)["HW_"] * 4 >= 2048)
    key = (tuple(sorted((k, v) for k, v in cfg.items())), pack_scores)
    if key not in _NC_CACHE:
        _NC_CACHE[key] = build(cfg, pack_scores=pack_scores)
    nc = _NC_CACHE[key]
    in_maps = [host_inputs(cfg, x, k_cache, v_cache, Wq, Wo, c)
               for c in range(8)]
    res = None
    for attempt in range(3):
        try:
            res = run_bass_kernel_spmd(nc, in_maps, core_ids=list(range(8)),
                                       trace=trace)
            break
        except Exception:
            if attempt == 2:
                raise
            time.sleep(2.0)
    outs = [res.results[c]["out"] for c in range(8)]
    return _assemble(cfg, outs), res


def kernel(x, k_cache, v_cache, Wq, Wo):
    full, _ = run(FULL, np.asarray(x), np.asarray(k_cache),
                  np.asarray(v_cache), np.asarray(Wq), np.asarray(Wo))
    return full.astype(np.float32)
